# revision 13
# baseline (speedup 1.0000x reference)
"""Causal Performer attention on 8 trn2 NeuronCores — bf16 pipeline.

Sharding: core c handles batch b = c // 4 and head-group hg = c % 4
(3 of the 12 heads). Each core:
  1. computes the qkv projection for its 3 heads (576 of 2304 rows),
  2. runs the causal linear-attention scan in chunked form (the
     (T,F,D) cumsum tensor is never materialized: intra-chunk masked
     (K'Q') scores plus an inter-chunk (F,D) running state),
  3. computes a partial output projection over its 192 channels.
The host sums the 4 partial (C,T) projections per batch and
transposes back to (B,T,C).

Numerics: everything SBUF-resident is bf16 (matmuls run 1 cycle/row at
any free size; DVE gets 2x on 2-byte ops; HBM traffic is halved); PSUM
accumulation stays fp32 and the final output is stored fp32. The
1/sqrt(F) on q', k' cancels in num/den; EPS is scaled by F.

Layouts:
  - Inputs arrive in 8 large DMAs (host pre-concatenates the k-tiles
    of x^T and W^T side by side) instead of 17 small ones — HWDGE
    config time (~0.6us per dma_start) dominated the startup phase.
  - stk_{q,k} per head: rows 0:64 raw (q|k), rows 64:128 squared —
    squares computed SBUF->SBUF in bf16 after a single psum copy.
  - q'^T/k'^T per head [F, T]: two partition-split prime matmuls into
    one [128,512] psum + two Exps (base-partition rules forbid mixing
    halves of one tile as matmul operands).
  - K' natural [t, f] obtained by PE-transposing k'^T (no second
    exp); packed 4 chunks per [S, 256] tile.
  - [V | 1] extended operand gives numerator+denominator in one
    matmul column block; the ones column is memset exactly once.
  - Scores for a chunk pair sit in one [S, 384] psum ([own0 | cross |
    own1]); a single [S,384] DVE multiply with the [tri|ones|tri]
    mask replaces three ops.
  - Division uses a [S,2] strided-AP extract of both denominators.

Scheduling: single PSUM pool with static tags (A projections/
transposes/yproj, B vdir/numden, C scores/knat, St scan state x3).
The scan runs pair-outer / head-inner so the three heads' state
chains interleave on the PE, and each output-projection half is
emitted as soon as its four chunk pairs exist — PE never waits on a
single head's serial state chain.
"""

import numpy as np
import ml_dtypes

import concourse.bacc as bacc
import concourse.bass as bass
import concourse.mybir as mybir
from concourse import tile
from concourse.bass_utils import run_bass_kernel_spmd

B, T, C = 2, 1024, 768
H, D, F = 12, 64, 64
HPC = 3  # heads per core
S = 128  # scan chunk length
NCH = T // S  # 8 chunks
CP = HPC * D  # 192 channels per core
N_CORES = 8
KT = C // 128  # 6 contraction tiles for the qkv matmul
WCOL = 3 * CP  # 576 qkv-weight columns per core
EPS_SCALED = float(F) * 1e-6  # compensates dropping 1/sqrt(F) on q', k'

FP32 = mybir.dt.float32
BF16 = mybir.dt.bfloat16
EXP = mybir.ActivationFunctionType.Exp
COPY = mybir.ActivationFunctionType.Copy


def build_program(n_iters=1):
    nc = bacc.Bacc(
        "TRN2", target_bir_lowering=False, debug=False, num_devices=N_CORES
    )
    # consts: [tri|ones|tri] mask (384) | ident (128) | projext (64)
    consts = nc.dram_tensor("consts", [128, 576], BF16, kind="ExternalInput").ap()
    xtb = nc.dram_tensor("xtb", [128, KT * T], BF16, kind="ExternalInput").ap()
    wtb = nc.dram_tensor("wtb", [128, KT * WCOL], BF16, kind="ExternalInput").ap()
    wpt = nc.dram_tensor("wpt", [CP, C], BF16, kind="ExternalInput").ap()
    yt = nc.dram_tensor("yt", [C, T], FP32, kind="ExternalOutput").ap()

    from contextlib import ExitStack

    with tile.TileContext(nc) as tc:
        with ExitStack() as ctx:
            pools = _make_pools(ctx, tc)
            for _ in range(n_iters):
                _body(pools, tc, consts, xtb, wtb, wpt, yt)
    nc.compile()
    return nc


def _make_pools(ctx, tc):
    return {
        # persistent tensors double-buffer across iterations so iteration
        # n+1's DMAs and front-end overlap iteration n's scan
        "big": ctx.enter_context(tc.tile_pool(name="big", bufs=2)),
        "psum": ctx.enter_context(tc.tile_pool(name="psum", bufs=2, space="PSUM")),
        "vnat_sb": ctx.enter_context(tc.tile_pool(name="vnat_sb", bufs=3)),
        "scan_sb": ctx.enter_context(tc.tile_pool(name="scan_sb", bufs=8)),
        "y_sb": ctx.enter_context(tc.tile_pool(name="y_sb", bufs=6)),
    }


def _body(pools, tc, consts, xtb, wtb, wpt, yt):
    nc = tc.nc

    big = pools["big"]
    const_s = big.tile([128, 576], BF16, name="consts", tag="consts")
    nc.sync.dma_start(const_s[:], consts)
    mask_s = const_s[:, 0 : 3 * S]
    ident_s = const_s[:, 3 * S : 3 * S + 128]
    projext_s = const_s[:, 3 * S + 128 : 3 * S + 192]

    # inputs in few big DMAs, split across the two HWDGE queues
    # (sync=SP gets x, scalar=ACT gets weights); wpt last
    xtall = big.tile([128, KT * T], BF16, name="xtall", tag="xtall")
    for g in range(3):
        nc.sync.dma_start(
            xtall[:, g * 2 * T : (g + 1) * 2 * T],
            xtb[:, g * 2 * T : (g + 1) * 2 * T],
        )
    xt_s = [xtall[:, k * T : (k + 1) * T] for k in range(KT)]
    wtall = big.tile([128, KT * WCOL], BF16, name="wtall", tag="wtall")
    for g in range(2):
        nc.scalar.dma_start(
            wtall[:, g * 3 * WCOL : (g + 1) * 3 * WCOL],
            wtb[:, g * 3 * WCOL : (g + 1) * 3 * WCOL],
        )
    wt_s = [wtall[:, k * WCOL : (k + 1) * WCOL] for k in range(KT)]
    wpt_a = big.tile([128, C], BF16, name="wpt_a", tag="wpt_a")
    nc.scalar.dma_start(wpt_a[:], wpt[0:128, :])
    wpt_b = big.tile([CP - 128, C], BF16, name="wpt_b", tag="wpt_b")
    nc.scalar.dma_start(wpt_b[:], wpt[128:CP, :])

    # single PSUM pool, static tags: A (projections/transposes/yproj) x2,
    # BC (vdir/numden/scores/knat) x3, St (scan state, one per head) x3
    psum = pools["psum"]

    # persistent SBUF tensors
    stk = {}  # (kind, h): rows 0:64 = (q|k) head h, rows 64:128 squared
    qpT = {}  # h: [F, T] q'^T
    kpT = {}  # h: [F, T] k'^T
    for h in range(HPC):
        for kind in "qk":
            stk[(kind, h)] = big.tile(
                [128, T], BF16, name=f"stk{kind}{h}", tag=f"stk{kind}{h}"
            )
        qpT[h] = big.tile([F, T], BF16, name=f"qpT{h}", tag=f"qpT{h}")
        kpT[h] = big.tile([F, T], BF16, name=f"kpT{h}", tag=f"kpT{h}")
    vext = {}  # (h, chunk) -> [S, D+1], col D = 1.0 (memset once)
    for h in range(HPC):
        for i in range(NCH):
            vext[(h, i)] = big.tile(
                [S, D + 1], BF16, name=f"vx{h}_{i}", tag=f"vx{h}_{i}"
            )
            nc.gpsimd.memset(vext[(h, i)][:, D : D + 1], 1.0)
    knatg = {}  # (h, g) -> [S, 256]: chunks 4g..4g+3 in col quarters
    for h in range(HPC):
        for g in range(2):
            knatg[(h, g)] = big.tile(
                [S, 256], BF16, name=f"kn{h}_{g}", tag=f"kn{h}_{g}"
            )
    outT01 = big.tile([128, T], BF16, name="outT01", tag="outT01")
    outT2 = big.tile([D, T], BF16, name="outT2", tag="outT2")

    vdir_chunks_done = [0]
    vnat_sb = pools["vnat_sb"]

    def emit_vdir(upto):
        # V in natural [t, d] layout via its own matmul (wt cols 384:576)
        while vdir_chunks_done[0] < upto:
            i = vdir_chunks_done[0]
            pt = psum.tile([S, CP], FP32, name="vdp", tag="BC", bufs=3)
            for k in range(KT):
                nc.tensor.matmul(
                    pt[:],
                    xt_s[k][:, i * S : (i + 1) * S],
                    wt_s[k][:, 2 * CP : 3 * CP],
                    start=(k == 0),
                    stop=(k == KT - 1),
                )
            vnat = vnat_sb.tile([S, CP], BF16, name="vnat", tag="vnat")
            if i % 2 == 0:
                nc.vector.tensor_copy(vnat[:], pt[:])
            else:
                nc.scalar.activation(vnat[:], pt[:], COPY)
            for h in range(HPC):
                nc.gpsimd.tensor_copy(
                    vext[(h, i)][:, 0:D], vnat[:, h * D : (h + 1) * D]
                )
            vdir_chunks_done[0] += 1

    # --- per-head front-end: qkv M-tile (q_h | k_h), raw copies, SBUF
    # squares, partition-split prime matmuls + Exp, K'nat via transpose ---
    for h in range(HPC):
        pts = [
            psum.tile([128, 512], FP32, name=f"qkvp{h}{nh}", tag="A")
            for nh in range(2)
        ]
        for k in range(KT):
            lhs = wt_s[k][:, h * 128 : (h + 1) * 128]
            for nh in range(2):
                nc.tensor.matmul(
                    pts[nh][:],
                    lhs,
                    xt_s[k][:, nh * 512 : (nh + 1) * 512],
                    start=(k == 0),
                    stop=(k == KT - 1),
                )
        for nh in range(2):
            cols = slice(nh * 512, (nh + 1) * 512)
            sq, sk = stk[("q", h)], stk[("k", h)]
            nc.scalar.activation(sq[0:64, cols], pts[nh][0:64, :], COPY)
            nc.vector.tensor_copy(sk[0:64, cols], pts[nh][64:128, :])
            # squares SBUF->SBUF bf16 (2x DVE)
            nc.vector.tensor_mul(sq[64:128, cols], sq[0:64, cols], sq[0:64, cols])
            nc.vector.tensor_mul(sk[64:128, cols], sk[0:64, cols], sk[0:64, cols])

        if h == 0:
            emit_vdir(2)

        # q'^T/k'^T: two partition-split matmuls into one psum, two Exps
        for nh in range(2):
            cols = slice(nh * 512, (nh + 1) * 512)
            pp = psum.tile([128, 512], FP32, name="pp", tag="A")
            nc.tensor.matmul(
                pp[0:64, :], projext_s[:], stk[("q", h)][:, cols],
                start=True, stop=True,
            )
            nc.tensor.matmul(
                pp[64:128, :], projext_s[:], stk[("k", h)][:, cols],
                start=True, stop=True,
            )
            nc.scalar.activation(qpT[h][:, cols], pp[0:64, :], EXP)
            nc.scalar.activation(kpT[h][:, cols], pp[64:128, :], EXP)

        # K' natural [t, f] by transposing k'^T; 4 chunks per psum tile
        for g in range(2):
            tp = psum.tile([128, 256], BF16, name="knp", tag="BC", bufs=3)
            for j in range(4):
                i = 4 * g + j
                nc.tensor.transpose(
                    tp[:, j * 64 : (j + 1) * 64],
                    kpT[h][:, i * S : (i + 1) * S],
                    ident_s[0:64, 0:64],
                )
            if (h + g) % 2 == 0:
                nc.vector.tensor_copy(knatg[(h, g)][:], tp[:])
            else:
                nc.scalar.activation(knatg[(h, g)][:], tp[:], COPY)

        emit_vdir(2 * (h + 1) + 2)

    emit_vdir(NCH)

    # --- scan: pair-outer / head-inner; chunk pair (2p, 2p+1) scores in
    # one [S,384] psum; state advances once per pair per head ---
    sb = pools["scan_sb"]
    ysb = pools["y_sb"]
    emit_yproj = _yproj_maker(nc, psum, ysb, wpt_a, wpt_b, outT01, outT2, yt)
    och_pair = {}
    och2 = {}
    for i in range(NCH):
        och_pair[i] = sb.tile([S, 128], BF16, name=f"ochp{i}", tag="ochp", bufs=4)
        och2[i] = sb.tile([S, D], BF16, name=f"och2_{i}", tag="och2", bufs=4)
    state_ps = {
        h: psum.tile([F, D + 1], FP32, name=f"state{h}", tag="St", bufs=3)[:]
        for h in range(HPC)
    }

    for p in range(NCH // 2):
        i0, i1 = 2 * p, 2 * p + 1
        c0 = slice(i0 * S, (i0 + 1) * S)
        c1 = slice(i1 * S, (i1 + 1) * S)
        cpair = slice(i0 * S, (i0 + 2) * S)

        for h in range(HPC):
            if p > 0:
                ssb = sb.tile([F, D + 1], BF16, name="ssb", tag="ssb")
                if (h + p) % 2 == 0:
                    nc.vector.tensor_copy(ssb[:], state_ps[h])
                else:
                    nc.scalar.activation(ssb[:], state_ps[h], COPY)

            # packed scores: [K'0 x Q'(0|1) | K'1 x Q'1]
            stp = psum.tile([S, 384], FP32, name="stp", tag="BC", bufs=3)
            nc.tensor.matmul(
                stp[:, 0:256], kpT[h][:, c0], qpT[h][:, cpair],
                start=True, stop=True,
            )
            nc.tensor.matmul(
                stp[:, 256:384], kpT[h][:, c1], qpT[h][:, c1],
                start=True, stop=True,
            )
            stm = sb.tile([S, 384], BF16, name="stm", tag="stm")
            nc.vector.tensor_mul(stm[:], stp[:], mask_s[:])

            # packed numden, sequential groups (a bank's zero region is
            # lazily zeroed on write: close group 0 before group 1 opens)
            ndp = psum.tile([S, 2 * (D + 1)], FP32, name="ndp", tag="BC", bufs=3)
            nd0 = ndp[:, 0 : D + 1]
            nd1 = ndp[:, D + 1 : 2 * (D + 1)]
            nc.tensor.matmul(
                nd0, stm[:, 0:128], vext[(h, i0)][:], start=True, stop=(p == 0)
            )
            if p > 0:
                nc.tensor.matmul(
                    nd0, qpT[h][:, c0], ssb[:], start=False, stop=True
                )
            nc.tensor.matmul(
                nd1, stm[:, 256:384], vext[(h, i1)][:], start=True, stop=False
            )
            nc.tensor.matmul(
                nd1, stm[:, 128:256], vext[(h, i0)][:], start=False, stop=(p == 0)
            )
            if p > 0:
                nc.tensor.matmul(
                    nd1, qpT[h][:, c1], ssb[:], start=False, stop=True
                )

            # packed division: [S,2] strided denominator extract
            dinv = sb.tile([S, 2], FP32, name="dinv", tag="dinv")
            nc.scalar.activation(
                dinv[:], ndp[:, D : 2 * (D + 1) : D + 1], COPY, bias=EPS_SCALED
            )
            nc.vector.reciprocal(dinv[:], dinv[:])
            for idx, i in enumerate((i0, i1)):
                och = (
                    och_pair[i][:, h * D : (h + 1) * D] if h < 2 else och2[i][:]
                )
                src = ndp[:, idx * (D + 1) : idx * (D + 1) + D]
                dv = dinv[:, idx : idx + 1]
                if (h + i) % 2 == 0:
                    nc.scalar.activation(och, src, COPY, scale=dv)
                else:
                    nc.vector.tensor_scalar_mul(och, src, dv)

            # state += K'^T [V | 1] for both chunks of the pair
            for i in (i0, i1):
                nc.tensor.matmul(
                    state_ps[h],
                    knatg[(h, i // 4)][:, (i % 4) * 64 : (i % 4 + 1) * 64],
                    vext[(h, i)][:],
                    start=(i == 0),
                    stop=True,
                    skip_group_check=True,
                )

        # transposes once all three heads' divisions for the pair exist
        tp = psum.tile([128, 256], BF16, name="tp", tag="A")
        nc.tensor.transpose(tp[:, 0:128], och_pair[i0][:], ident_s[:, 0:128])
        nc.tensor.transpose(tp[:, 128:256], och_pair[i1][:], ident_s[:, 0:128])
        if p % 2 == 0:
            nc.vector.tensor_copy(outT01[:, cpair], tp[:])
        else:
            nc.scalar.activation(outT01[:, cpair], tp[:], COPY)
        tp2 = psum.tile([D, 256], BF16, name="tp2", tag="A")
        nc.tensor.transpose(tp2[:, 0:128], och2[i0][:], ident_s[:, 0:128])
        nc.tensor.transpose(tp2[:, 128:256], och2[i1][:], ident_s[:, 0:128])
        if p % 2 == 0:
            nc.scalar.activation(outT2[:, cpair], tp2[:], COPY)
        else:
            nc.vector.tensor_copy(outT2[:, cpair], tp2[:])
        if p % 2 == 1:
            emit_yproj(p // 2)


def _yproj_maker(nc, psum, ysb, wpt_a, wpt_b, outT01, outT2, yt):
    def emit_yproj(nh):
        cols = slice(nh * 512, (nh + 1) * 512)
        for ot in range(C // 128):
            ypt = psum.tile([128, 512], FP32, name="ypt", tag="A")
            nc.tensor.matmul(
                ypt[:],
                wpt_a[:, ot * 128 : (ot + 1) * 128],
                outT01[:, cols],
                start=True,
                stop=False,
            )
            nc.tensor.matmul(
                ypt[:],
                wpt_b[:, ot * 128 : (ot + 1) * 128],
                outT2[:, cols],
                start=False,
                stop=True,
            )
            yo = ysb.tile([128, 512], FP32, name="yo", tag="yo")
            if ot % 2 == 0:
                nc.vector.tensor_copy(yo[:], ypt[:])
            else:
                nc.scalar.activation(yo[:], ypt[:], COPY)
            nc.sync.dma_start(yt[ot * 128 : (ot + 1) * 128, cols], yo[:])

    return emit_yproj


_PROGRAM = None


def _get_program():
    global _PROGRAM
    if _PROGRAM is None:
        _PROGRAM = build_program()
    return _PROGRAM


def _bf16(a):
    return np.ascontiguousarray(a).astype(ml_dtypes.bfloat16)


def make_core_inputs(x, W_attn, W_proj, proj, core):
    b, hg = divmod(core, 4)
    heads = list(range(HPC * hg, HPC * (hg + 1)))
    rows = []
    for h in heads:  # (q_h | k_h) pairs, then the v block
        rows.extend(range(h * D, (h + 1) * D))
        rows.extend(range(C + h * D, C + (h + 1) * D))
    for h in heads:
        rows.extend(range(2 * C + h * D, 2 * C + (h + 1) * D))
    projext = np.concatenate(
        [proj.astype(np.float32), np.full((D, F), -0.5, np.float32)], axis=0
    )
    tri = np.triu(np.ones((S, S), np.float32))
    consts = np.concatenate(
        [tri, np.ones((S, S), np.float32), tri,
         np.eye(128, dtype=np.float32), projext],
        axis=1,
    )
    xt = np.ascontiguousarray(x[b].T)  # [C, T]
    xtb = np.concatenate(
        [xt[k * 128 : (k + 1) * 128] for k in range(KT)], axis=1
    )
    wt = np.ascontiguousarray(W_attn[rows, :].T)  # [C, 576]
    wtb = np.concatenate(
        [wt[k * 128 : (k + 1) * 128] for k in range(KT)], axis=1
    )
    return {
        "consts": _bf16(consts),
        "xtb": _bf16(xtb),
        "wtb": _bf16(wtb),
        "wpt": _bf16(W_proj[:, CP * hg : CP * (hg + 1)].T),
    }


def kernel(x, W_attn, W_proj, proj):
    nc = _get_program()
    in_maps = [
        make_core_inputs(x, W_attn, W_proj, proj, core) for core in range(N_CORES)
    ]
    res = run_bass_kernel_spmd(nc, in_maps, list(range(N_CORES)))
    out = np.empty((B, T, C), np.float32)
    for b in range(B):
        acc = res.results[4 * b]["yt"].astype(np.float32).copy()
        for g in range(1, 4):
            acc += res.results[4 * b + g]["yt"]
        out[b] = acc.T
    return out


# revision 14
# speedup vs baseline: 1.6385x; 1.6385x over previous
"""Causal Performer attention on 8 trn2 NeuronCores — bf16 pipeline.

Sharding: core c handles batch b = c // 4 and head-group hg = c % 4
(3 of the 12 heads). Each core:
  1. computes the qkv projection for its 3 heads (576 of 2304 rows),
  2. runs the causal linear-attention scan in chunked form (the
     (T,F,D) cumsum tensor is never materialized: intra-chunk masked
     (K'Q') scores plus an inter-chunk (F,D) running state),
  3. computes a partial output projection over its 192 channels.
The host sums the 4 partial (C,T) projections per batch and
transposes back to (B,T,C).

Numerics: everything SBUF-resident is bf16 (matmuls run 1 cycle/row at
any free size; DVE gets 2x on 2-byte ops; HBM traffic is halved); PSUM
accumulation stays fp32 and the final output is stored fp32. The
1/sqrt(F) on q', k' cancels in num/den; EPS is scaled by F.

Layouts:
  - Inputs arrive in 8 large DMAs (host pre-concatenates the k-tiles
    of x^T and W^T side by side) instead of 17 small ones — HWDGE
    config time (~0.6us per dma_start) dominated the startup phase.
  - stk_{q,k} per head: rows 0:64 raw (q|k), rows 64:128 squared —
    squares computed SBUF->SBUF in bf16 after a single psum copy.
  - q'^T/k'^T per head [F, T]: two partition-split prime matmuls into
    one [128,512] psum + two Exps (base-partition rules forbid mixing
    halves of one tile as matmul operands).
  - K' natural [t, f] obtained by PE-transposing k'^T (no second
    exp); packed 4 chunks per [S, 256] tile.
  - [V | 1] extended operand gives numerator+denominator in one
    matmul column block; the ones column is memset exactly once.
  - Scores for a chunk pair sit in one [S, 384] psum ([own0 | cross |
    own1]); a single [S,384] DVE multiply with the [tri|ones|tri]
    mask replaces three ops.
  - Division uses a [S,2] strided-AP extract of both denominators.

Scheduling: single PSUM pool with static tags (A projections/
transposes/yproj, B vdir/numden, C scores/knat, St scan state x3).
The scan runs pair-outer / head-inner so the three heads' state
chains interleave on the PE, and each output-projection half is
emitted as soon as its four chunk pairs exist — PE never waits on a
single head's serial state chain.
"""

import numpy as np
import ml_dtypes

import concourse.bacc as bacc
import concourse.bass as bass
import concourse.mybir as mybir
from concourse import tile
from concourse.bass_utils import run_bass_kernel_spmd

B, T, C = 2, 1024, 768
H, D, F = 12, 64, 64
HPC = 3  # heads per core
S = 128  # scan chunk length
NCH = T // S  # 8 chunks
CP = HPC * D  # 192 channels per core
N_CORES = 8
KT = C // 128  # 6 contraction tiles for the qkv matmul
WCOL = 3 * CP  # 576 qkv-weight columns per core
EPS_SCALED = float(F) * 1e-6  # compensates dropping 1/sqrt(F) on q', k'

FP32 = mybir.dt.float32
BF16 = mybir.dt.bfloat16
EXP = mybir.ActivationFunctionType.Exp
COPY = mybir.ActivationFunctionType.Copy


def build_program(n_iters=1):
    nc = bacc.Bacc(
        "TRN2", target_bir_lowering=False, debug=False, num_devices=N_CORES
    )
    # consts: [tri|ones|tri] mask (384) | ident (128) | projext (64)
    consts = nc.dram_tensor("consts", [128, 576], BF16, kind="ExternalInput").ap()
    xtb = nc.dram_tensor("xtb", [128, KT * T], BF16, kind="ExternalInput").ap()
    wtb = nc.dram_tensor("wtb", [128, KT * WCOL], BF16, kind="ExternalInput").ap()
    wpt = nc.dram_tensor("wpt", [CP, C], BF16, kind="ExternalInput").ap()
    yt = nc.dram_tensor("yt", [C, T], FP32, kind="ExternalOutput").ap()

    from contextlib import ExitStack

    with tile.TileContext(nc) as tc:
        for _ in range(n_iters):
            with ExitStack() as ctx:
                pools = _make_pools(ctx, tc)
                _body(pools, tc, consts, xtb, wtb, wpt, yt)
    nc.compile()
    return nc


def _make_pools(ctx, tc):
    return {
        "big": ctx.enter_context(tc.tile_pool(name="big", bufs=1)),
        "psum": ctx.enter_context(tc.tile_pool(name="psum", bufs=2, space="PSUM")),
        "vnat_sb": ctx.enter_context(tc.tile_pool(name="vnat_sb", bufs=3)),
        "scan_sb": ctx.enter_context(tc.tile_pool(name="scan_sb", bufs=8)),
        "y_sb": ctx.enter_context(tc.tile_pool(name="y_sb", bufs=6)),
    }


def _body(pools, tc, consts, xtb, wtb, wpt, yt):
    nc = tc.nc

    big = pools["big"]
    const_s = big.tile([128, 576], BF16, name="consts", tag="consts")
    nc.sync.dma_start(const_s[:], consts)
    mask_s = const_s[:, 0 : 3 * S]
    ident_s = const_s[:, 3 * S : 3 * S + 128]
    projext_s = const_s[:, 3 * S + 128 : 3 * S + 192]

    # inputs in few big DMAs, split across the two HWDGE queues
    # (sync=SP gets x, scalar=ACT gets weights); wpt last
    xtall = big.tile([128, KT * T], BF16, name="xtall", tag="xtall")
    for g in range(3):
        nc.sync.dma_start(
            xtall[:, g * 2 * T : (g + 1) * 2 * T],
            xtb[:, g * 2 * T : (g + 1) * 2 * T],
        )
    xt_s = [xtall[:, k * T : (k + 1) * T] for k in range(KT)]
    wtall = big.tile([128, KT * WCOL], BF16, name="wtall", tag="wtall")
    for g in range(2):
        nc.scalar.dma_start(
            wtall[:, g * 3 * WCOL : (g + 1) * 3 * WCOL],
            wtb[:, g * 3 * WCOL : (g + 1) * 3 * WCOL],
        )
    wt_s = [wtall[:, k * WCOL : (k + 1) * WCOL] for k in range(KT)]
    wpt_a = big.tile([128, C], BF16, name="wpt_a", tag="wpt_a")
    nc.scalar.dma_start(wpt_a[:], wpt[0:128, :])
    wpt_b = big.tile([CP - 128, C], BF16, name="wpt_b", tag="wpt_b")
    nc.scalar.dma_start(wpt_b[:], wpt[128:CP, :])

    # single PSUM pool, static tags: A (projections/transposes/yproj) x2,
    # BC (vdir/numden/scores/knat) x3, St (scan state, one per head) x3
    psum = pools["psum"]

    # persistent SBUF tensors
    stk = {}  # (kind, h): rows 0:64 = (q|k) head h, rows 64:128 squared
    qpT = {}  # h: [F, T] q'^T
    kpT = {}  # h: [F, T] k'^T
    for h in range(HPC):
        for kind in "qk":
            stk[(kind, h)] = big.tile(
                [128, T], BF16, name=f"stk{kind}{h}", tag=f"stk{kind}{h}"
            )
        qpT[h] = big.tile([F, T], BF16, name=f"qpT{h}", tag=f"qpT{h}")
        kpT[h] = big.tile([F, T], BF16, name=f"kpT{h}", tag=f"kpT{h}")
    vext = {}  # (h, chunk) -> [S, D+1], col D = 1.0 (memset once)
    for h in range(HPC):
        for i in range(NCH):
            vext[(h, i)] = big.tile(
                [S, D + 1], BF16, name=f"vx{h}_{i}", tag=f"vx{h}_{i}"
            )
            nc.gpsimd.memset(vext[(h, i)][:, D : D + 1], 1.0)
    knatg = {}  # (h, g) -> [S, 256]: chunks 4g..4g+3 in col quarters
    for h in range(HPC):
        for g in range(2):
            knatg[(h, g)] = big.tile(
                [S, 256], BF16, name=f"kn{h}_{g}", tag=f"kn{h}_{g}"
            )
    outT01 = big.tile([128, T], BF16, name="outT01", tag="outT01")
    outT2 = big.tile([D, T], BF16, name="outT2", tag="outT2")

    vdir_chunks_done = [0]
    vnat_sb = pools["vnat_sb"]

    def emit_vdir(upto):
        # V in natural [t, d] layout via its own matmul (wt cols 384:576)
        while vdir_chunks_done[0] < upto:
            i = vdir_chunks_done[0]
            pt = psum.tile([S, CP], FP32, name="vdp", tag="BC", bufs=3)
            for k in range(KT):
                nc.tensor.matmul(
                    pt[:],
                    xt_s[k][:, i * S : (i + 1) * S],
                    wt_s[k][:, 2 * CP : 3 * CP],
                    start=(k == 0),
                    stop=(k == KT - 1),
                )
            vnat = vnat_sb.tile([S, CP], BF16, name="vnat", tag="vnat")
            if i % 2 == 0:
                nc.vector.tensor_copy(vnat[:], pt[:])
            else:
                nc.scalar.activation(vnat[:], pt[:], COPY)
            for h in range(HPC):
                nc.gpsimd.tensor_copy(
                    vext[(h, i)][:, 0:D], vnat[:, h * D : (h + 1) * D]
                )
            vdir_chunks_done[0] += 1

    # --- per-head front-end: qkv M-tile (q_h | k_h), raw copies, SBUF
    # squares, partition-split prime matmuls + Exp, K'nat via transpose ---
    for h in range(HPC):
        pts = [
            psum.tile([128, 512], FP32, name=f"qkvp{h}{nh}", tag="A")
            for nh in range(2)
        ]
        for k in range(KT):
            lhs = wt_s[k][:, h * 128 : (h + 1) * 128]
            for nh in range(2):
                nc.tensor.matmul(
                    pts[nh][:],
                    lhs,
                    xt_s[k][:, nh * 512 : (nh + 1) * 512],
                    start=(k == 0),
                    stop=(k == KT - 1),
                )
        for nh in range(2):
            cols = slice(nh * 512, (nh + 1) * 512)
            sq, sk = stk[("q", h)], stk[("k", h)]
            nc.scalar.activation(sq[0:64, cols], pts[nh][0:64, :], COPY)
            nc.vector.tensor_copy(sk[0:64, cols], pts[nh][64:128, :])
            # squares SBUF->SBUF bf16 (2x DVE)
            nc.vector.tensor_mul(sq[64:128, cols], sq[0:64, cols], sq[0:64, cols])
            nc.vector.tensor_mul(sk[64:128, cols], sk[0:64, cols], sk[0:64, cols])

        if h == 0:
            emit_vdir(2)

        # q'^T/k'^T: two partition-split matmuls into one psum, two Exps
        for nh in range(2):
            cols = slice(nh * 512, (nh + 1) * 512)
            pp = psum.tile([128, 512], FP32, name="pp", tag="A")
            nc.tensor.matmul(
                pp[0:64, :], projext_s[:], stk[("q", h)][:, cols],
                start=True, stop=True,
            )
            nc.tensor.matmul(
                pp[64:128, :], projext_s[:], stk[("k", h)][:, cols],
                start=True, stop=True,
            )
            nc.scalar.activation(qpT[h][:, cols], pp[0:64, :], EXP)
            nc.scalar.activation(kpT[h][:, cols], pp[64:128, :], EXP)

        # K' natural [t, f] by transposing k'^T; 4 chunks per psum tile
        for g in range(2):
            tp = psum.tile([128, 256], BF16, name="knp", tag="BC", bufs=3)
            for j in range(4):
                i = 4 * g + j
                nc.tensor.transpose(
                    tp[:, j * 64 : (j + 1) * 64],
                    kpT[h][:, i * S : (i + 1) * S],
                    ident_s[0:64, 0:64],
                )
            if (h + g) % 2 == 0:
                nc.vector.tensor_copy(knatg[(h, g)][:], tp[:])
            else:
                nc.scalar.activation(knatg[(h, g)][:], tp[:], COPY)

        emit_vdir(2 * (h + 1) + 2)

    emit_vdir(NCH)

    # --- scan: pair-outer / head-inner; chunk pair (2p, 2p+1) scores in
    # one [S,384] psum; state advances once per pair per head ---
    sb = pools["scan_sb"]
    ysb = pools["y_sb"]
    emit_yproj = _yproj_maker(nc, psum, ysb, wpt_a, wpt_b, outT01, outT2, yt)
    och_pair = {}
    och2 = {}
    for i in range(NCH):
        och_pair[i] = sb.tile([S, 128], BF16, name=f"ochp{i}", tag="ochp", bufs=4)
        och2[i] = sb.tile([S, D], BF16, name=f"och2_{i}", tag="och2", bufs=4)
    state_ps = {
        h: psum.tile([F, D + 1], FP32, name=f"state{h}", tag="St", bufs=3)[:]
        for h in range(HPC)
    }

    for p in range(NCH // 2):
        i0, i1 = 2 * p, 2 * p + 1
        c0 = slice(i0 * S, (i0 + 1) * S)
        c1 = slice(i1 * S, (i1 + 1) * S)
        cpair = slice(i0 * S, (i0 + 2) * S)

        for h in range(HPC):
            if p > 0:
                ssb = sb.tile([F, D + 1], BF16, name="ssb", tag="ssb")
                if (h + p) % 2 == 0:
                    nc.vector.tensor_copy(ssb[:], state_ps[h])
                else:
                    nc.scalar.activation(ssb[:], state_ps[h], COPY)

            # packed scores: [K'0 x Q'(0|1) | K'1 x Q'1]
            stp = psum.tile([S, 384], FP32, name="stp", tag="BC", bufs=3)
            nc.tensor.matmul(
                stp[:, 0:256], kpT[h][:, c0], qpT[h][:, cpair],
                start=True, stop=True,
            )
            nc.tensor.matmul(
                stp[:, 256:384], kpT[h][:, c1], qpT[h][:, c1],
                start=True, stop=True,
            )
            stm = sb.tile([S, 384], BF16, name="stm", tag="stm")
            nc.vector.tensor_mul(stm[:], stp[:], mask_s[:])

            # packed numden, sequential groups (a bank's zero region is
            # lazily zeroed on write: close group 0 before group 1 opens)
            ndp = psum.tile([S, 2 * (D + 1)], FP32, name="ndp", tag="BC", bufs=3)
            nd0 = ndp[:, 0 : D + 1]
            nd1 = ndp[:, D + 1 : 2 * (D + 1)]
            nc.tensor.matmul(
                nd0, stm[:, 0:128], vext[(h, i0)][:], start=True, stop=(p == 0)
            )
            if p > 0:
                nc.tensor.matmul(
                    nd0, qpT[h][:, c0], ssb[:], start=False, stop=True
                )
            nc.tensor.matmul(
                nd1, stm[:, 256:384], vext[(h, i1)][:], start=True, stop=False
            )
            nc.tensor.matmul(
                nd1, stm[:, 128:256], vext[(h, i0)][:], start=False, stop=(p == 0)
            )
            if p > 0:
                nc.tensor.matmul(
                    nd1, qpT[h][:, c1], ssb[:], start=False, stop=True
                )

            # packed division: [S,2] strided denominator extract
            dinv = sb.tile([S, 2], FP32, name="dinv", tag="dinv")
            nc.scalar.activation(
                dinv[:], ndp[:, D : 2 * (D + 1) : D + 1], COPY, bias=EPS_SCALED
            )
            nc.vector.reciprocal(dinv[:], dinv[:])
            for idx, i in enumerate((i0, i1)):
                och = (
                    och_pair[i][:, h * D : (h + 1) * D] if h < 2 else och2[i][:]
                )
                src = ndp[:, idx * (D + 1) : idx * (D + 1) + D]
                dv = dinv[:, idx : idx + 1]
                if (h + i) % 2 == 0:
                    nc.scalar.activation(och, src, COPY, scale=dv)
                else:
                    nc.vector.tensor_scalar_mul(och, src, dv)

            # state += K'^T [V | 1] for both chunks of the pair
            for i in (i0, i1):
                nc.tensor.matmul(
                    state_ps[h],
                    knatg[(h, i // 4)][:, (i % 4) * 64 : (i % 4 + 1) * 64],
                    vext[(h, i)][:],
                    start=(i == 0),
                    stop=True,
                    skip_group_check=True,
                )

        # transposes once all three heads' divisions for the pair exist
        tp = psum.tile([128, 256], BF16, name="tp", tag="A")
        nc.tensor.transpose(tp[:, 0:128], och_pair[i0][:], ident_s[:, 0:128])
        nc.tensor.transpose(tp[:, 128:256], och_pair[i1][:], ident_s[:, 0:128])
        if p % 2 == 0:
            nc.vector.tensor_copy(outT01[:, cpair], tp[:])
        else:
            nc.scalar.activation(outT01[:, cpair], tp[:], COPY)
        tp2 = psum.tile([D, 256], BF16, name="tp2", tag="A")
        nc.tensor.transpose(tp2[:, 0:128], och2[i0][:], ident_s[:, 0:128])
        nc.tensor.transpose(tp2[:, 128:256], och2[i1][:], ident_s[:, 0:128])
        if p % 2 == 0:
            nc.scalar.activation(outT2[:, cpair], tp2[:], COPY)
        else:
            nc.vector.tensor_copy(outT2[:, cpair], tp2[:])
        if p % 2 == 1:
            emit_yproj(p // 2)


def _yproj_maker(nc, psum, ysb, wpt_a, wpt_b, outT01, outT2, yt):
    def emit_yproj(nh):
        cols = slice(nh * 512, (nh + 1) * 512)
        for ot in range(C // 128):
            ypt = psum.tile([128, 512], FP32, name="ypt", tag="A")
            nc.tensor.matmul(
                ypt[:],
                wpt_a[:, ot * 128 : (ot + 1) * 128],
                outT01[:, cols],
                start=True,
                stop=False,
            )
            nc.tensor.matmul(
                ypt[:],
                wpt_b[:, ot * 128 : (ot + 1) * 128],
                outT2[:, cols],
                start=False,
                stop=True,
            )
            yo = ysb.tile([128, 512], FP32, name="yo", tag="yo")
            if ot % 2 == 0:
                nc.vector.tensor_copy(yo[:], ypt[:])
            else:
                nc.scalar.activation(yo[:], ypt[:], COPY)
            nc.sync.dma_start(yt[ot * 128 : (ot + 1) * 128, cols], yo[:])

    return emit_yproj


_PROGRAM = None


def _get_program():
    global _PROGRAM
    if _PROGRAM is None:
        _PROGRAM = build_program()
    return _PROGRAM


def _bf16(a):
    return np.ascontiguousarray(a).astype(ml_dtypes.bfloat16)


def make_core_inputs(x, W_attn, W_proj, proj, core):
    b, hg = divmod(core, 4)
    heads = list(range(HPC * hg, HPC * (hg + 1)))
    rows = []
    for h in heads:  # (q_h | k_h) pairs, then the v block
        rows.extend(range(h * D, (h + 1) * D))
        rows.extend(range(C + h * D, C + (h + 1) * D))
    for h in heads:
        rows.extend(range(2 * C + h * D, 2 * C + (h + 1) * D))
    projext = np.concatenate(
        [proj.astype(np.float32), np.full((D, F), -0.5, np.float32)], axis=0
    )
    tri = np.triu(np.ones((S, S), np.float32))
    consts = np.concatenate(
        [tri, np.ones((S, S), np.float32), tri,
         np.eye(128, dtype=np.float32), projext],
        axis=1,
    )
    xt = np.ascontiguousarray(x[b].T)  # [C, T]
    xtb = np.concatenate(
        [xt[k * 128 : (k + 1) * 128] for k in range(KT)], axis=1
    )
    wt = np.ascontiguousarray(W_attn[rows, :].T)  # [C, 576]
    wtb = np.concatenate(
        [wt[k * 128 : (k + 1) * 128] for k in range(KT)], axis=1
    )
    return {
        "consts": _bf16(consts),
        "xtb": _bf16(xtb),
        "wtb": _bf16(wtb),
        "wpt": _bf16(W_proj[:, CP * hg : CP * (hg + 1)].T),
    }


def kernel(x, W_attn, W_proj, proj):
    nc = _get_program()
    in_maps = [
        make_core_inputs(x, W_attn, W_proj, proj, core) for core in range(N_CORES)
    ]
    res = run_bass_kernel_spmd(nc, in_maps, list(range(N_CORES)))
    out = np.empty((B, T, C), np.float32)
    for b in range(B):
        acc = res.results[4 * b]["yt"].astype(np.float32).copy()
        for g in range(1, 4):
            acc += res.results[4 * b + g]["yt"]
        out[b] = acc.T
    return out
